# revision 18
# baseline (speedup 1.0000x reference)
"""Trainium2 Bass kernel for packed-varlen causal attention (16 heads, D=1024).

Strategy: data-parallel over segments across 8 NeuronCores. Each core packs
1-2 segments tile-aligned into a 1536-token buffer. One SPMD program; all
per-core differences are data (packed inputs only).

v2 redesign vs baseline:
- Causal masking is ADDITIVE, applied on the PE: for diagonal tile pairs a
  constant strictly-upper-triangular -30000 matrix is accumulated into the
  score PSUM via one extra matmul (lhsT=diagA, rhs=I). Off-diagonal tiles
  need no mask at all (segments are tile-aligned; padded keys only exist in
  a segment's last tile where the causal mask kills them since pad position
  > any valid query position in that tile). This removes the [TOK,TOK] mask
  DMA and all mask-multiply vector traffic.
- exp() is batched: scores for several k-tiles are packed side by side into
  one [128, <=1024] PSUM tile (2 banks) and activated in a single ACTIVATE,
  writing bf16 e-tiles consumed directly by the PV matmuls.
- Per-head loop. Softmax denominators come from the ones-column appended to
  V (as before); 1/den uses reciprocal_approx_fast directly on the PSUM row
  (the baseline's 48x4us RECIPROCALs were 24% of runtime), broadcast across
  64 partitions on the idle GpSimd engine.
- RoPE runs in bf16 (DVE 2x mode) with the final add on GpSimd.
"""
import os
from contextlib import ExitStack

import numpy as np
import ml_dtypes

import concourse.bass as bass
import concourse.tile as tile
from concourse import bacc, mybir
from concourse.bass_utils import run_bass_kernel_spmd

BF16 = ml_dtypes.bfloat16
F32 = np.float32
NCORES = 8
NT = 12            # query tiles of 128 -> 1536 token slots per core
TOK = NT * 128
EMBED, HEADS, HDIM = 1024, 16, 64
DT = mybir.dt
NEG = -30000.0

LAST_EXEC_NS = None
LAST_TRACE = None
_CACHE = {}


def _install_ntff_shim():
    """Provide antenv.axon_hooks (missing in this image) so
    run_bass_kernel_spmd(trace=True) can capture NTFF profiles via the
    axon .so, and keep artifacts local instead of uploading."""
    import sys
    import types
    try:
        import antenv.axon_hooks  # noqa: F401
        return
    except ImportError:
        pass
    try:
        from trn_agent_boot.trn_boot import _ntff_profile_via_ctypes
        hook = _ntff_profile_via_ctypes("/opt/axon/libaxon_pjrt.so")
    except Exception:
        hook = None
    mod = types.ModuleType("antenv.axon_hooks")
    mod.get_axon_ntff_profile_hook = lambda: hook
    mod.set_axon_ntff_profile_hook = lambda h: None
    sys.modules["antenv.axon_hooks"] = mod
    import concourse.bass_utils as _bu
    _bu.upload_artifacts = lambda tmpdir: tmpdir


# ---------------------------------------------------------------- planning --

def _build_plan(seq_lens):
    segs = sorted(range(len(seq_lens)), key=lambda i: -int(seq_lens[i]))
    loads = [0.0] * NCORES
    tiles_used = [0] * NCORES
    assign = [[] for _ in range(NCORES)]
    for s in segs:
        L = int(seq_lens[s])
        nt = (L + 127) // 128
        cost = L * 8.4e6 + (L * L) * 2048.0
        placed = False
        for c in sorted(range(NCORES), key=lambda c: loads[c]):
            if tiles_used[c] + nt <= NT:
                assign[c].append(s)
                loads[c] += cost
                tiles_used[c] += nt
                placed = True
                break
        assert placed, "segments do not fit the 8x1536 structure"
    core_chunks = []
    for c in range(NCORES):
        t0, chunks = 0, []
        for s in assign[c]:
            L = int(seq_lens[s])
            chunks.append((s, t0, L))
            t0 += (L + 127) // 128
        core_chunks.append(chunks)
    pairs = set()
    for chunks in core_chunks:
        for (_, t0, L) in chunks:
            nt = (L + 127) // 128
            for a in range(nt):
                for b in range(a + 1):
                    pairs.add((t0 + a, t0 + b))
    # tile pairs that cross a segment boundary on SOME core (the SPMD union
    # executes them everywhere; such cores must kill them with a rank-1
    # additive mask fed by per-core bmaskT data)
    cross = set()
    for chunks in core_chunks:
        seg_of = {}
        for si, (_, t0, L) in enumerate(chunks):
            for t in range(t0, t0 + (L + 127) // 128):
                seg_of[t] = si
        for (qi, kj) in pairs:
            if qi in seg_of and kj in seg_of and seg_of[qi] != seg_of[kj]:
                cross.add((qi, kj))
    # per 512-col chunk (cch): klist entries -> bank-packed exp groups
    structure = []
    bmoff = [0]
    for cch in range(3):
        entries = []
        for kj in range(NT):
            qs = [qi for (qi, k2) in pairs
                  if k2 == kj and 4 * cch <= qi < 4 * cch + 4]
            if qs:
                entries.append((kj, min(qs), max(qs) + 1))
        # pack entries into 512-col PSUM banks
        banks = []
        for ent in sorted(entries, key=lambda e: -(e[2] - e[1])):
            w = (ent[2] - ent[1]) * 128
            for b in banks:
                if b[0] + w <= 512:
                    b[0] += w
                    b[1].append(ent)
                    break
            else:
                banks.append([w, [ent]])
        fulls = [b for b in banks if b[0] == 512]
        parts = [b for b in banks if b[0] < 512]
        bs = fulls + parts
        groups = []
        i = 0
        while i < len(bs):
            if i + 1 < len(bs) and bs[i][0] == 512:
                groups.append([bs[i], bs[i + 1]])
                i += 2
            else:
                groups.append([bs[i]])
                i += 1
        # flatten each group to (off, kj, qlo, qhi, diag_off, nq, bm)
        gl = []
        for g in groups:
            flat = []
            off = 0
            for bank_i, (_, ents) in enumerate(g):
                off = bank_i * 512
                for (kj, qlo, qhi) in ents:
                    nq = (qhi - qlo) * 128
                    doff = (kj - qlo) * 128 if qlo <= kj < qhi else None
                    if any((qi, kj) in cross for qi in range(qlo, qhi)):
                        bm = bmoff[0]
                        bmoff[0] += nq
                    else:
                        bm = None
                    flat.append((off, kj, qlo, qhi, doff, nq, bm))
                    off += nq
            gcols = flat[-1][0] + flat[-1][5]
            gl.append((gcols, tuple(flat)))
        structure.append(tuple(gl))
    return core_chunks, (tuple(structure), max(bmoff[0], 128))


# ---------------------------------------------------------- device program --

def _emit_program(structure_and_bm):
    structure, bmcols = structure_and_bm
    nc = bacc.Bacc("TRN2", target_bir_lowering=False, debug=False,
                   num_devices=NCORES)
    f32, bf16 = DT.float32, DT.bfloat16
    EXP = mybir.ActivationFunctionType.Exp
    IDN = mybir.ActivationFunctionType.Identity

    xT_d = nc.dram_tensor("xT", [EMBED, TOK], bf16, kind="ExternalInput").ap()
    cosT_d = nc.dram_tensor("cosT", [128, TOK], bf16, kind="ExternalInput").ap()
    sinT_d = nc.dram_tensor("sinT", [128, TOK], bf16, kind="ExternalInput").ap()
    wq_d = nc.dram_tensor("wqT", [EMBED, EMBED], bf16, kind="ExternalInput").ap()
    wk_d = nc.dram_tensor("wkT", [EMBED, EMBED], bf16, kind="ExternalInput").ap()
    wv_d = nc.dram_tensor("wvT", [EMBED, EMBED], bf16, kind="ExternalInput").ap()
    wo_d = nc.dram_tensor("woT", [EMBED, EMBED], bf16, kind="ExternalInput").ap()
    qbT_d = nc.dram_tensor("qbT", [128, 8], f32, kind="ExternalInput").ap()
    obT_d = nc.dram_tensor("obT", [128, 8], f32, kind="ExternalInput").ap()
    vb_d = nc.dram_tensor("vb", [1, EMBED], bf16, kind="ExternalInput").ap()
    diag_d = nc.dram_tensor("diagA", [128, 256], bf16, kind="ExternalInput").ap()
    bm_d = nc.dram_tensor("bmaskT", [1, bmcols], bf16, kind="ExternalInput").ap()
    yT_d = nc.dram_tensor("yT", [EMBED, TOK], f32, kind="ExternalOutput").ap()

    with tile.TileContext(nc) as tc, ExitStack() as ctx:
        singles = ctx.enter_context(tc.tile_pool(name="singles", bufs=1))
        wpool = ctx.enter_context(tc.tile_pool(name="wpool", bufs=2))
        persist = ctx.enter_context(tc.tile_pool(name="persist", bufs=1))
        # PSUM: sb slots (3 x 2 banks, shared by proj acc / scores / outproj)
        # + pa (2 x 1 bank) = 8 banks
        spool = ctx.enter_context(tc.tile_pool(name="spool", bufs=3, space="PSUM"))
        papool = ctx.enter_context(tc.tile_pool(name="papool", bufs=2, space="PSUM"))

        # constants / persistent tensors
        qb_sb = singles.tile([128, 8], f32, tag="qb")
        nc.sync.dma_start(out=qb_sb, in_=qbT_d)
        ob_sb = singles.tile([128, 8], f32, tag="ob")
        nc.sync.dma_start(out=ob_sb, in_=obT_d)
        vb_sb = singles.tile([1, EMBED], bf16, tag="vb")
        nc.sync.dma_start(out=vb_sb, in_=vb_d)
        diag_sb = singles.tile([128, 256], bf16, tag="diag")
        nc.sync.dma_start(out=diag_sb, in_=diag_d)
        bm_sb = singles.tile([1, bmcols], bf16, tag="bm")
        nc.sync.dma_start(out=bm_sb, in_=bm_d)
        ones_sb = singles.tile([1, 512], bf16, tag="ones")
        nc.vector.memset(ones_sb, 1.0)
        negrow = singles.tile([1, 128], bf16, tag="negrow")
        nc.vector.memset(negrow, NEG)

        qr_sb = persist.tile([128, 8, TOK], bf16, tag="qr")
        kr_sb = persist.tile([128, 8, TOK], bf16, tag="kr")
        # v with a ones column appended per head: [tok_tile, head, 65]
        va_sb = persist.tile([128, NT, HEADS, HDIM + 1], bf16, tag="va")
        nc.vector.memset(va_sb[:, :, :, HDIM:HDIM + 1], 1.0)

        def load_w(dram):
            w = wpool.tile([128, 8, EMBED], bf16, tag="w")
            r = dram.rearrange("(a p) n -> p a n", p=128)
            for ab in range(4):
                nc.sync.dma_start(out=w[:, 2 * ab:2 * ab + 2, :],
                                  in_=r[:, 2 * ab:2 * ab + 2, :])
            return w

        # ----------------------------------------------- projections + RoPE
        xpool = ctx.enter_context(tc.tile_pool(name="xpool", bufs=1))
        x_sb = xpool.tile([128, 8, TOK], bf16, tag="x")
        for c3 in range(3):
            nc.sync.dma_start(
                out=x_sb[:, :, bass.ts(c3, 512)],
                in_=xT_d[:, bass.ts(c3, 512)].rearrange("(a p) t -> p a t",
                                                        p=128))
        with tc.tile_pool(name="cspool", bufs=1) as cspool, \
             tc.tile_pool(name="rope", bufs=2) as rope:
            cos_sb = cspool.tile([128, TOK], bf16, tag="cos")
            nc.sync.dma_start(out=cos_sb, in_=cosT_d)
            sin_sb = cspool.tile([128, TOK], bf16, tag="sin")
            nc.sync.dma_start(out=sin_sb, in_=sinT_d)

            def rope_proj(w_sb, bias_cols, out_sb):
                for m in range(8):
                    for cpair in ((0, 1), (2,)):
                        w = 512 * len(cpair)
                        t0c = cpair[0] * 512
                        tw = bass.ds(t0c, w)
                        ps = spool.tile([128, w], f32, tag="sb")
                        for ci in range(len(cpair)):
                            for a in range(8):
                                nc.tensor.matmul(
                                    ps[:, ci * 512:ci * 512 + 512],
                                    w_sb[:, a, bass.ts(m, 128)],
                                    x_sb[:, a, bass.ts(cpair[ci], 512)],
                                    start=(a == 0), stop=(a == 7),
                                    skip_group_check=True)
                        qc = rope.tile([128, w], bf16, tag="qc")
                        if bias_cols is not None:
                            nc.scalar.activation(qc, ps, IDN,
                                                 bias=bias_cols[:, m:m + 1])
                        else:
                            nc.scalar.copy(qc, ps)
                        sw = rope.tile([128, w], bf16, tag="sw")
                        for half in range(2):
                            b = half * 64
                            nc.sync.dma_start(out=sw[b:b + 32, :],
                                              in_=qc[b + 32:b + 64, :])
                            nc.sync.dma_start(out=sw[b + 32:b + 64, :],
                                              in_=qc[b:b + 32, :])
                        m1 = rope.tile([128, w], bf16, tag="m1")
                        nc.vector.tensor_mul(m1, qc, cos_sb[:, tw])
                        m2 = rope.tile([128, w], bf16, tag="m2")
                        nc.vector.tensor_mul(m2, sw, sin_sb[:, tw])
                        nc.gpsimd.tensor_add(out_sb[:, m, tw], m1, m2)

            wq = load_w(wq_d)
            rope_proj(wq, qb_sb, qr_sb)
            wk = load_w(wk_d)
            rope_proj(wk, None, kr_sb)
            wv = load_w(wv_d)

            def v_piece(tt):
                ps = spool.tile([128, 1024], f32, tag="sb")
                for n2 in range(2):
                    pshalf = ps[:, n2 * 512:n2 * 512 + 512]
                    for a in range(8):
                        nc.tensor.matmul(pshalf, x_sb[:, a, bass.ts(tt, 128)],
                                         wv[:, a, bass.ts(n2, 512)],
                                         start=(a == 0), stop=False,
                                         skip_group_check=True)
                    nc.tensor.matmul(pshalf, ones_sb[:, 0:128],
                                     vb_sb[:, bass.ts(n2, 512)], start=False,
                                     stop=True, skip_group_check=True)
                nc.scalar.copy(va_sb[:, tt, :, 0:HDIM],
                               ps.rearrange("p (h d) -> p h d", d=HDIM))

            def v_half(tt, n2):
                def f():
                    ps = spool.tile([128, 512], f32, tag="sb")
                    for a in range(8):
                        nc.tensor.matmul(ps, x_sb[:, a, bass.ts(tt, 128)],
                                         wv[:, a, bass.ts(n2, 512)],
                                         start=(a == 0), stop=False,
                                         skip_group_check=True)
                    nc.tensor.matmul(ps, ones_sb[:, 0:128],
                                     vb_sb[:, bass.ts(n2, 512)], start=False,
                                     stop=True, skip_group_check=True)
                    nc.scalar.copy(va_sb[:, tt, bass.ts(n2, 8), 0:HDIM],
                                   ps.rearrange("p (h d) -> p h d", d=HDIM))
                return f

            # v for tiles 0-3 now; tiles 4-11 are interleaved into the cch0
            # attention stream (not needed until attn cch1/cch2)
            for tt in range(4):
                v_piece(tt)
            wo_sb = load_w(wo_d)

        # ------------------------------------------------------- attention --
        with tc.tile_pool(name="epool", bufs=3) as epool, \
             tc.tile_pool(name="rpool", bufs=2) as rpool, \
             tc.tile_pool(name="brpool", bufs=2) as brpool, \
             tc.tile_pool(name="attnp", bufs=2) as attnp, \
             tc.tile_pool(name="ypool", bufs=2) as ypool:
            attn_tiles = {}

            def out_piece(cch, m):
                attn_sb = attn_tiles[cch]
                py = spool.tile([128, 512], f32, tag="sb")
                for r in range(8):
                    nc.tensor.matmul(py, wo_sb[:, r, bass.ts(m, 128)],
                                     attn_sb[:, r, :], start=(r == 0),
                                     stop=(r == 7))
                ys = ypool.tile([128, 512], f32, tag="ys")
                nc.scalar.activation(ys, py, IDN, bias=ob_sb[:, m:m + 1])
                nc.sync.dma_start(
                    out=yT_d[bass.ts(m, 128), bass.ts(cch, 512)], in_=ys)

            for cch in range(3):
                q0 = cch * 512
                groups = structure[cch]
                nent_total = sum(len(g[1]) for g in groups)
                attn_sb = attnp.tile([128, 8, 512], bf16, tag="attn")
                attn_tiles[cch] = attn_sb
                oj = 0
                for h in range(HEADS):
                    # interleave full-duty filler (deferred v-projection for
                    # cch0, previous chunk's out-projection for cch1/2) to
                    # keep the PE array duty (and HAM clock) up
                    if cch == 0 and h < 16:
                        v_half(4 + h // 2, h % 2)()
                    if cch > 0 and h % 2 == 0 and oj < 8:
                        out_piece(cch - 1, oj)
                        oj += 1
                    hh, hg = (h % 2) * 64, h // 2
                    pa = papool.tile([HDIM + 1, 512], f32, tag="pa")
                    nent = 0
                    for (gcols, flat) in groups:
                        sb = spool.tile([128, gcols], f32, tag="sb")
                        # per 512-col bank: QK(+masks), exp, then PV — PV of
                        # bank b starts while bank b+1 is still in QK/exp
                        for b0 in range(0, gcols, 512):
                            bank = [f for f in flat if b0 <= f[0] < b0 + 512]
                            bcols = max(f[0] + f[5] for f in bank) - b0
                            for (off, kj, qlo, qhi, doff, nq, bm) in bank:
                                krs = kr_sb[hh:hh + 64, hg, bass.ts(kj, 128)]
                                qrs = qr_sb[hh:hh + 64, hg,
                                            bass.ds(qlo * 128, nq)]
                                nc.tensor.matmul(sb[:, off:off + nq], krs, qrs,
                                                 start=True,
                                                 stop=(doff is None
                                                       and bm is None),
                                                 skip_group_check=True)
                                if bm is not None:
                                    nc.tensor.matmul(
                                        sb[:, off:off + nq], negrow,
                                        bm_sb[0:1, bass.ds(bm, nq)],
                                        start=False, stop=(doff is None),
                                        skip_group_check=True)
                                if doff is not None:
                                    nc.tensor.matmul(
                                        sb[:, off + doff:off + doff + 128],
                                        diag_sb[:, 0:128], diag_sb[:, 128:256],
                                        start=False, stop=True,
                                        skip_group_check=True)
                            e = epool.tile([128, bcols], bf16, tag="e")
                            nc.scalar.activation(e, sb[:, b0:b0 + bcols], EXP,
                                                 scale=0.125)
                            for (off, kj, qlo, qhi, doff, nq, bm) in bank:
                                qoff = qlo * 128 - q0
                                nc.tensor.matmul(
                                    pa[:, bass.ds(qoff, nq)],
                                    va_sb[:, kj, h, :],
                                    e[:, off - b0:off - b0 + nq],
                                    start=(nent == 0),
                                    stop=(nent == nent_total - 1),
                                    skip_group_check=True)
                                nent += 1
                    den = rpool.tile([1, 512], f32, tag="den")
                    nc.vector.tensor_copy(den, pa[HDIM:HDIM + 1, :])
                    rcp = rpool.tile([1, 512], f32, tag="rcp")
                    nc.vector.reciprocal_approx_fast(rcp, den)
                    rbs = brpool.tile([64, 512], f32, tag="rbs")
                    nc.gpsimd.partition_broadcast(rbs, rcp, channels=64)
                    nc.vector.tensor_mul(attn_sb[hh:hh + 64, hg, :],
                                         pa[0:HDIM, :], rbs)
                while cch > 0 and oj < 8:
                    out_piece(cch - 1, oj)
                    oj += 1
            for m in range(8):
                out_piece(2, m)
    nc.compile()
    return nc


# ------------------------------------------------------------- host driver --

def _host_prep(hidden, cos, sin, seq_lens, core_chunks, structure_and_bm):
    structure, bmcols = structure_and_bm
    starts = np.concatenate([[0], np.cumsum(seq_lens)]).astype(np.int64)
    per_core = []
    sgn = np.concatenate([-np.ones(32, F32), np.ones(32, F32)])
    for c in range(NCORES):
        tokmap = np.full(TOK, -1, np.int64)
        tseg = np.full(NT, -1, np.int64)
        for si, (s, t0, L) in enumerate(core_chunks[c]):
            sl = slice(t0 * 128, t0 * 128 + L)
            tokmap[sl] = np.arange(starts[s], starts[s] + L)
            tseg[t0:t0 + (L + 127) // 128] = si
        real = tokmap >= 0
        x = np.zeros((TOK, EMBED), F32)
        x[real] = hidden[tokmap[real]]
        cs = np.zeros((TOK, HDIM), F32)
        sn = np.zeros((TOK, HDIM), F32)
        cs[real] = cos[tokmap[real]]
        sn[real] = sin[tokmap[real]]
        cosT = np.tile(np.ascontiguousarray(cs.T), (2, 1)).astype(BF16)
        sinT = np.tile(np.ascontiguousarray(sn.T) * sgn[:, None],
                       (2, 1)).astype(BF16)
        # bmaskT[kj, q] = 1 where q's tile and kj belong to different segments
        # (both used); the device adds -30000 * bmask to those score columns.
        qt = np.arange(TOK) // 128
        bmask = ((tseg[:, None] != tseg[qt][None, :])
                 & (tseg[:, None] >= 0) & (tseg[qt][None, :] >= 0))
        bmrow = np.zeros((1, bmcols), F32)
        for groups in structure:
            for (_, flat) in groups:
                for (off, kj, qlo, qhi, doff, nq, bm) in flat:
                    if bm is not None:
                        bmrow[0, bm:bm + nq] = bmask[kj, qlo * 128:qlo * 128 + nq]
        per_core.append(dict(tokmap=tokmap,
                             xT=np.ascontiguousarray(x.T).astype(BF16),
                             cosT=cosT, sinT=sinT,
                             bmaskT=bmrow.astype(BF16)))
    return per_core


def _shared_inputs(q_w, q_b, k_w, v_w, v_b, out_w, out_b):
    diagA = np.zeros((128, 256), F32)
    diagA[:, 0:128] = np.triu(np.full((128, 128), NEG, F32), k=1)
    diagA[:, 128:256] = np.eye(128, dtype=F32)
    return {
        "wqT": np.ascontiguousarray(np.asarray(q_w, F32).T).astype(BF16),
        "wkT": np.ascontiguousarray(np.asarray(k_w, F32).T).astype(BF16),
        "wvT": np.ascontiguousarray(np.asarray(v_w, F32).T).astype(BF16),
        "woT": np.ascontiguousarray(np.asarray(out_w, F32).T).astype(BF16),
        "qbT": np.ascontiguousarray(np.asarray(q_b, F32).reshape(8, 128).T),
        "obT": np.ascontiguousarray(np.asarray(out_b, F32).reshape(8, 128).T),
        "vb": np.asarray(v_b, F32).reshape(1, EMBED).astype(BF16),
        "diagA": diagA.astype(BF16),
    }


def kernel(hidden_states, cos, sin, q_w, q_b, k_w, v_w, v_b, out_w, out_b,
           seq_len, max_seqlen):
    global LAST_EXEC_NS
    hidden = np.asarray(hidden_states, F32)
    cos = np.asarray(cos, F32)
    sin = np.asarray(sin, F32)
    seq_lens = [int(v) for v in np.asarray(seq_len)]

    core_chunks, structure = _build_plan(seq_lens)
    if structure not in _CACHE:
        _CACHE[structure] = _emit_program(structure)
    nc = _CACHE[structure]

    per_core = _host_prep(hidden, cos, sin, seq_lens, core_chunks, structure)
    shared = _shared_inputs(q_w, q_b, k_w, v_w, v_b, out_w, out_b)
    in_maps = []
    for c in range(NCORES):
        pc = per_core[c]
        in_maps.append({**shared, "xT": pc["xT"], "cosT": pc["cosT"],
                        "sinT": pc["sinT"], "bmaskT": pc["bmaskT"]})

    trace = os.environ.get("BASS_KERNEL_TRACE", "0") == "1"
    if trace:
        _install_ntff_shim()
    import time as _time
    _t0 = _time.time()
    res = run_bass_kernel_spmd(nc, in_maps, core_ids=list(range(NCORES)),
                               trace=trace)
    LAST_EXEC_NS = res.exec_time_ns
    globals()["LAST_TRACE"] = res.instructions_and_trace
    globals()["LAST_RUN_WALL_S"] = _time.time() - _t0

    T = hidden.shape[0]
    out = np.zeros((T, EMBED), F32)
    for c in range(NCORES):
        tokmap = per_core[c]["tokmap"]
        real = tokmap >= 0
        yT = np.asarray(res.results[c]["yT"], F32)
        out[tokmap[real]] = yT.T[real]
    return out


# revision 19
# speedup vs baseline: 1.2581x; 1.2581x over previous
"""Trainium2 Bass kernel for packed-varlen causal attention (16 heads, D=1024).

Strategy: data-parallel over segments across 8 NeuronCores. Each core packs
1-2 segments tile-aligned into a 1536-token buffer. One SPMD program; all
per-core differences are data (packed inputs only).

v2 redesign vs baseline:
- Causal masking is ADDITIVE, applied on the PE: for diagonal tile pairs a
  constant strictly-upper-triangular -30000 matrix is accumulated into the
  score PSUM via one extra matmul (lhsT=diagA, rhs=I). Off-diagonal tiles
  need no mask at all (segments are tile-aligned; padded keys only exist in
  a segment's last tile where the causal mask kills them since pad position
  > any valid query position in that tile). This removes the [TOK,TOK] mask
  DMA and all mask-multiply vector traffic.
- exp() is batched: scores for several k-tiles are packed side by side into
  one [128, <=1024] PSUM tile (2 banks) and activated in a single ACTIVATE,
  writing bf16 e-tiles consumed directly by the PV matmuls.
- Per-head loop. Softmax denominators come from the ones-column appended to
  V (as before); 1/den uses reciprocal_approx_fast directly on the PSUM row
  (the baseline's 48x4us RECIPROCALs were 24% of runtime), broadcast across
  64 partitions on the idle GpSimd engine.
- RoPE runs in bf16 (DVE 2x mode) with the final add on GpSimd.
"""
import os
from contextlib import ExitStack

import numpy as np
import ml_dtypes

import concourse.bass as bass
import concourse.tile as tile
from concourse import bacc, mybir
from concourse.bass_utils import run_bass_kernel_spmd

BF16 = ml_dtypes.bfloat16
F32 = np.float32
NCORES = 8
NT = 12            # query tiles of 128 -> 1536 token slots per core
TOK = NT * 128
EMBED, HEADS, HDIM = 1024, 16, 64
DT = mybir.dt
NEG = -30000.0

LAST_EXEC_NS = None
LAST_TRACE = None
_CACHE = {}


def _install_ntff_shim():
    """Provide antenv.axon_hooks (missing in this image) so
    run_bass_kernel_spmd(trace=True) can capture NTFF profiles via the
    axon .so, and keep artifacts local instead of uploading."""
    import sys
    import types
    try:
        import antenv.axon_hooks  # noqa: F401
        return
    except ImportError:
        pass
    try:
        from trn_agent_boot.trn_boot import _ntff_profile_via_ctypes
        hook = _ntff_profile_via_ctypes("/opt/axon/libaxon_pjrt.so")
    except Exception:
        hook = None
    mod = types.ModuleType("antenv.axon_hooks")
    mod.get_axon_ntff_profile_hook = lambda: hook
    mod.set_axon_ntff_profile_hook = lambda h: None
    sys.modules["antenv.axon_hooks"] = mod
    import concourse.bass_utils as _bu
    _bu.upload_artifacts = lambda tmpdir: tmpdir


# ---------------------------------------------------------------- planning --

def _build_plan(seq_lens):
    segs = sorted(range(len(seq_lens)), key=lambda i: -int(seq_lens[i]))
    loads = [0.0] * NCORES
    tiles_used = [0] * NCORES
    assign = [[] for _ in range(NCORES)]
    for s in segs:
        L = int(seq_lens[s])
        nt = (L + 127) // 128
        cost = L * 8.4e6 + (L * L) * 2048.0
        placed = False
        for c in sorted(range(NCORES), key=lambda c: loads[c]):
            if tiles_used[c] + nt <= NT:
                assign[c].append(s)
                loads[c] += cost
                tiles_used[c] += nt
                placed = True
                break
        assert placed, "segments do not fit the 8x1536 structure"
    core_chunks = []
    for c in range(NCORES):
        t0, chunks = 0, []
        for s in assign[c]:
            L = int(seq_lens[s])
            chunks.append((s, t0, L))
            t0 += (L + 127) // 128
        core_chunks.append(chunks)
    pairs = set()
    for chunks in core_chunks:
        for (_, t0, L) in chunks:
            nt = (L + 127) // 128
            for a in range(nt):
                for b in range(a + 1):
                    pairs.add((t0 + a, t0 + b))
    # tile pairs that cross a segment boundary on SOME core (the SPMD union
    # executes them everywhere; such cores must kill them with a rank-1
    # additive mask fed by per-core bmaskT data)
    cross = set()
    for chunks in core_chunks:
        seg_of = {}
        for si, (_, t0, L) in enumerate(chunks):
            for t in range(t0, t0 + (L + 127) // 128):
                seg_of[t] = si
        for (qi, kj) in pairs:
            if qi in seg_of and kj in seg_of and seg_of[qi] != seg_of[kj]:
                cross.add((qi, kj))
    # per 512-col chunk (cch): klist entries -> bank-packed exp groups
    structure = []
    bmoff = [0]
    for cch in range(3):
        entries = []
        for kj in range(NT):
            qs = [qi for (qi, k2) in pairs
                  if k2 == kj and 4 * cch <= qi < 4 * cch + 4]
            if qs:
                entries.append((kj, min(qs), max(qs) + 1))
        # pack entries into 512-col PSUM banks
        banks = []
        for ent in sorted(entries, key=lambda e: -(e[2] - e[1])):
            w = (ent[2] - ent[1]) * 128
            for b in banks:
                if b[0] + w <= 512:
                    b[0] += w
                    b[1].append(ent)
                    break
            else:
                banks.append([w, [ent]])
        fulls = [b for b in banks if b[0] == 512]
        parts = [b for b in banks if b[0] < 512]
        bs = fulls + parts
        groups = []
        i = 0
        while i < len(bs):
            if i + 1 < len(bs) and bs[i][0] == 512:
                groups.append([bs[i], bs[i + 1]])
                i += 2
            else:
                groups.append([bs[i]])
                i += 1
        # flatten each group to (off, kj, qlo, qhi, diag_off, nq, bm)
        gl = []
        for g in groups:
            flat = []
            off = 0
            for bank_i, (_, ents) in enumerate(g):
                off = bank_i * 512
                for (kj, qlo, qhi) in ents:
                    nq = (qhi - qlo) * 128
                    doff = (kj - qlo) * 128 if qlo <= kj < qhi else None
                    if any((qi, kj) in cross for qi in range(qlo, qhi)):
                        bm = bmoff[0]
                        bmoff[0] += nq
                    else:
                        bm = None
                    flat.append((off, kj, qlo, qhi, doff, nq, bm))
                    off += nq
            gcols = flat[-1][0] + flat[-1][5]
            gl.append((gcols, tuple(flat)))
        structure.append(tuple(gl))
    return core_chunks, (tuple(structure), max(bmoff[0], 128))


# ---------------------------------------------------------- device program --

def _emit_program(structure_and_bm):
    structure, bmcols = structure_and_bm
    nc = bacc.Bacc("TRN2", target_bir_lowering=False, debug=False,
                   num_devices=NCORES)
    f32, bf16 = DT.float32, DT.bfloat16
    EXP = mybir.ActivationFunctionType.Exp
    IDN = mybir.ActivationFunctionType.Identity

    xT_d = nc.dram_tensor("xT", [EMBED, TOK], bf16, kind="ExternalInput").ap()
    cosT_d = nc.dram_tensor("cosT", [128, TOK], bf16, kind="ExternalInput").ap()
    sinT_d = nc.dram_tensor("sinT", [128, TOK], bf16, kind="ExternalInput").ap()
    wq_d = nc.dram_tensor("wqT", [EMBED, EMBED], bf16, kind="ExternalInput").ap()
    wk_d = nc.dram_tensor("wkT", [EMBED, EMBED], bf16, kind="ExternalInput").ap()
    wv_d = nc.dram_tensor("wvT", [EMBED, EMBED], bf16, kind="ExternalInput").ap()
    wo_d = nc.dram_tensor("woT", [EMBED, EMBED], bf16, kind="ExternalInput").ap()
    qbT_d = nc.dram_tensor("qbT", [128, 8], f32, kind="ExternalInput").ap()
    obT_d = nc.dram_tensor("obT", [128, 8], f32, kind="ExternalInput").ap()
    vb_d = nc.dram_tensor("vb", [1, EMBED], bf16, kind="ExternalInput").ap()
    diag_d = nc.dram_tensor("diagA", [128, 256], bf16, kind="ExternalInput").ap()
    bm_d = nc.dram_tensor("bmaskT", [1, bmcols], bf16, kind="ExternalInput").ap()
    yT_d = nc.dram_tensor("yT", [EMBED, TOK], f32, kind="ExternalOutput").ap()

    with tile.TileContext(nc) as tc, ExitStack() as ctx:
        singles = ctx.enter_context(tc.tile_pool(name="singles", bufs=1))
        wpool = ctx.enter_context(tc.tile_pool(name="wpool", bufs=2))
        persist = ctx.enter_context(tc.tile_pool(name="persist", bufs=1))
        # PSUM: sb slots (3 x 2 banks, shared by proj acc / scores / outproj)
        # + pa (2 x 1 bank) = 8 banks
        spool = ctx.enter_context(tc.tile_pool(name="spool", bufs=3, space="PSUM"))
        papool = ctx.enter_context(tc.tile_pool(name="papool", bufs=2, space="PSUM"))

        # constants / persistent tensors
        qb_sb = singles.tile([128, 8], f32, tag="qb")
        nc.sync.dma_start(out=qb_sb, in_=qbT_d)
        ob_sb = singles.tile([128, 8], f32, tag="ob")
        nc.sync.dma_start(out=ob_sb, in_=obT_d)
        vb_sb = singles.tile([1, EMBED], bf16, tag="vb")
        nc.sync.dma_start(out=vb_sb, in_=vb_d)
        diag_sb = singles.tile([128, 256], bf16, tag="diag")
        nc.sync.dma_start(out=diag_sb, in_=diag_d)
        bm_sb = singles.tile([1, bmcols], bf16, tag="bm")
        nc.sync.dma_start(out=bm_sb, in_=bm_d)
        ones_sb = singles.tile([1, 512], bf16, tag="ones")
        nc.vector.memset(ones_sb, 1.0)
        negrow = singles.tile([1, 128], bf16, tag="negrow")
        nc.vector.memset(negrow, NEG)

        qr_sb = persist.tile([128, 8, TOK], bf16, tag="qr")
        kr_sb = persist.tile([128, 8, TOK], bf16, tag="kr")
        # v with a ones column appended per head: [tok_tile, head, 65]
        va_sb = persist.tile([128, NT, HEADS, HDIM + 1], bf16, tag="va")
        nc.vector.memset(va_sb[:, :, :, HDIM:HDIM + 1], 1.0)

        def load_w(dram):
            w = wpool.tile([128, 8, EMBED], bf16, tag="w")
            r = dram.rearrange("(a p) n -> p a n", p=128)
            for ab in range(4):
                nc.sync.dma_start(out=w[:, 2 * ab:2 * ab + 2, :],
                                  in_=r[:, 2 * ab:2 * ab + 2, :])
            return w

        # ----------------------------------------------- projections + RoPE
        xpool = ctx.enter_context(tc.tile_pool(name="xpool", bufs=1))
        x_sb = xpool.tile([128, 8, TOK], bf16, tag="x")
        for c3 in range(3):
            nc.sync.dma_start(
                out=x_sb[:, :, bass.ts(c3, 512)],
                in_=xT_d[:, bass.ts(c3, 512)].rearrange("(a p) t -> p a t",
                                                        p=128))
        with tc.tile_pool(name="cspool", bufs=1) as cspool, \
             tc.tile_pool(name="rope", bufs=2) as rope:
            cos_sb = cspool.tile([128, TOK], bf16, tag="cos")
            nc.sync.dma_start(out=cos_sb, in_=cosT_d)
            sin_sb = cspool.tile([128, TOK], bf16, tag="sin")
            nc.sync.dma_start(out=sin_sb, in_=sinT_d)

            def rope_proj(w_sb, bias_cols, out_sb):
                for m in range(8):
                    for cpair in ((0, 1), (2,)):
                        w = 512 * len(cpair)
                        t0c = cpair[0] * 512
                        tw = bass.ds(t0c, w)
                        ps = spool.tile([128, w], f32, tag="sb")
                        for ci in range(len(cpair)):
                            for a in range(8):
                                nc.tensor.matmul(
                                    ps[:, ci * 512:ci * 512 + 512],
                                    w_sb[:, a, bass.ts(m, 128)],
                                    x_sb[:, a, bass.ts(cpair[ci], 512)],
                                    start=(a == 0), stop=(a == 7),
                                    skip_group_check=True)
                        qc = rope.tile([128, w], bf16, tag="qc")
                        if bias_cols is not None:
                            nc.scalar.activation(qc, ps, IDN,
                                                 bias=bias_cols[:, m:m + 1])
                        else:
                            nc.scalar.copy(qc, ps)
                        sw = rope.tile([128, w], bf16, tag="sw")
                        for half in range(2):
                            b = half * 64
                            nc.sync.dma_start(out=sw[b:b + 32, :],
                                              in_=qc[b + 32:b + 64, :])
                            nc.sync.dma_start(out=sw[b + 32:b + 64, :],
                                              in_=qc[b:b + 32, :])
                        m1 = rope.tile([128, w], bf16, tag="m1")
                        nc.vector.tensor_mul(m1, qc, cos_sb[:, tw])
                        m2 = rope.tile([128, w], bf16, tag="m2")
                        nc.vector.tensor_mul(m2, sw, sin_sb[:, tw])
                        nc.gpsimd.tensor_add(out_sb[:, m, tw], m1, m2)

            wq = load_w(wq_d)
            rope_proj(wq, qb_sb, qr_sb)
            wk = load_w(wk_d)
            rope_proj(wk, None, kr_sb)
            wv = load_w(wv_d)

            def v_piece(tt):
                ps = spool.tile([128, 1024], f32, tag="sb")
                for n2 in range(2):
                    pshalf = ps[:, n2 * 512:n2 * 512 + 512]
                    for a in range(8):
                        nc.tensor.matmul(pshalf, x_sb[:, a, bass.ts(tt, 128)],
                                         wv[:, a, bass.ts(n2, 512)],
                                         start=(a == 0), stop=False,
                                         skip_group_check=True)
                    nc.tensor.matmul(pshalf, ones_sb[:, 0:128],
                                     vb_sb[:, bass.ts(n2, 512)], start=False,
                                     stop=True, skip_group_check=True)
                nc.scalar.copy(va_sb[:, tt, :, 0:HDIM],
                               ps.rearrange("p (h d) -> p h d", d=HDIM))

            def v_half(tt, n2):
                def f():
                    ps = spool.tile([128, 512], f32, tag="sb")
                    for a in range(8):
                        nc.tensor.matmul(ps, x_sb[:, a, bass.ts(tt, 128)],
                                         wv[:, a, bass.ts(n2, 512)],
                                         start=(a == 0), stop=False,
                                         skip_group_check=True)
                    nc.tensor.matmul(ps, ones_sb[:, 0:128],
                                     vb_sb[:, bass.ts(n2, 512)], start=False,
                                     stop=True, skip_group_check=True)
                    nc.scalar.copy(va_sb[:, tt, bass.ts(n2, 8), 0:HDIM],
                                   ps.rearrange("p (h d) -> p h d", d=HDIM))
                return f

            # v for tiles 0-3 now; tiles 4-11 are interleaved into the cch0
            # attention stream (not needed until attn cch1/cch2)
            for tt in range(4):
                v_piece(tt)
            wo_sb = load_w(wo_d)

        # ------------------------------------------------------- attention --
        with tc.tile_pool(name="epool", bufs=3) as epool, \
             tc.tile_pool(name="rpool", bufs=2) as rpool, \
             tc.tile_pool(name="brpool", bufs=2) as brpool, \
             tc.tile_pool(name="attnp", bufs=2) as attnp, \
             tc.tile_pool(name="ypool", bufs=2) as ypool:
            attn_tiles = {}

            def out_piece(cch, m):
                attn_sb = attn_tiles[cch]
                py = spool.tile([128, 512], f32, tag="sb")
                for r in range(8):
                    nc.tensor.matmul(py, wo_sb[:, r, bass.ts(m, 128)],
                                     attn_sb[:, r, :], start=(r == 0),
                                     stop=(r == 7))
                ys = ypool.tile([128, 512], f32, tag="ys")
                nc.scalar.activation(ys, py, IDN, bias=ob_sb[:, m:m + 1])
                nc.sync.dma_start(
                    out=yT_d[bass.ts(m, 128), bass.ts(cch, 512)], in_=ys)

            for cch in range(3):
                q0 = cch * 512
                groups = structure[cch]
                nent_total = sum(len(g[1]) for g in groups)
                attn_sb = attnp.tile([128, 8, 512], bf16, tag="attn")
                attn_tiles[cch] = attn_sb
                oj = 0
                for h in range(HEADS):
                    # interleave full-duty filler (deferred v-projection for
                    # cch0, previous chunk's out-projection for cch1/2) to
                    # keep the PE array duty (and HAM clock) up
                    if cch == 0 and h < 16:
                        v_half(4 + h // 2, h % 2)()
                    if cch > 0 and h % 2 == 0 and oj < 8:
                        out_piece(cch - 1, oj)
                        oj += 1
                    hh, hg = (h % 2) * 64, h // 2
                    pa = papool.tile([HDIM + 1, 512], f32, tag="pa")
                    nent = 0
                    for (gcols, flat) in groups:
                        sb = spool.tile([128, gcols], f32, tag="sb")
                        for (off, kj, qlo, qhi, doff, nq, bm) in flat:
                            krs = kr_sb[hh:hh + 64, hg, bass.ts(kj, 128)]
                            qrs = qr_sb[hh:hh + 64, hg, bass.ds(qlo * 128, nq)]
                            nc.tensor.matmul(sb[:, off:off + nq], krs, qrs,
                                             start=True,
                                             stop=(doff is None and bm is None),
                                             skip_group_check=True)
                            if bm is not None:
                                nc.tensor.matmul(
                                    sb[:, off:off + nq], negrow,
                                    bm_sb[0:1, bass.ds(bm, nq)],
                                    start=False, stop=(doff is None),
                                    skip_group_check=True)
                            if doff is not None:
                                nc.tensor.matmul(
                                    sb[:, off + doff:off + doff + 128],
                                    diag_sb[:, 0:128], diag_sb[:, 128:256],
                                    start=False, stop=True,
                                    skip_group_check=True)
                        e = epool.tile([128, gcols], bf16, tag="e")
                        nc.scalar.activation(e, sb, EXP, scale=0.125)
                        for (off, kj, qlo, qhi, doff, nq, bm) in flat:
                            qoff = qlo * 128 - q0
                            nc.tensor.matmul(
                                pa[:, bass.ds(qoff, nq)],
                                va_sb[:, kj, h, :], e[:, off:off + nq],
                                start=(nent == 0),
                                stop=(nent == nent_total - 1),
                                skip_group_check=True)
                            nent += 1
                    den = rpool.tile([1, 512], f32, tag="den")
                    nc.vector.tensor_copy(den, pa[HDIM:HDIM + 1, :])
                    rcp = rpool.tile([1, 512], f32, tag="rcp")
                    nc.vector.reciprocal_approx_fast(rcp, den)
                    rbs = brpool.tile([64, 512], f32, tag="rbs")
                    nc.gpsimd.partition_broadcast(rbs, rcp, channels=64)
                    nc.vector.tensor_mul(attn_sb[hh:hh + 64, hg, :],
                                         pa[0:HDIM, :], rbs)
                while cch > 0 and oj < 8:
                    out_piece(cch - 1, oj)
                    oj += 1
            for m in range(8):
                out_piece(2, m)
    nc.compile()
    return nc


# ------------------------------------------------------------- host driver --

def _host_prep(hidden, cos, sin, seq_lens, core_chunks, structure_and_bm):
    structure, bmcols = structure_and_bm
    starts = np.concatenate([[0], np.cumsum(seq_lens)]).astype(np.int64)
    per_core = []
    sgn = np.concatenate([-np.ones(32, F32), np.ones(32, F32)])
    for c in range(NCORES):
        tokmap = np.full(TOK, -1, np.int64)
        tseg = np.full(NT, -1, np.int64)
        for si, (s, t0, L) in enumerate(core_chunks[c]):
            sl = slice(t0 * 128, t0 * 128 + L)
            tokmap[sl] = np.arange(starts[s], starts[s] + L)
            tseg[t0:t0 + (L + 127) // 128] = si
        real = tokmap >= 0
        x = np.zeros((TOK, EMBED), F32)
        x[real] = hidden[tokmap[real]]
        cs = np.zeros((TOK, HDIM), F32)
        sn = np.zeros((TOK, HDIM), F32)
        cs[real] = cos[tokmap[real]]
        sn[real] = sin[tokmap[real]]
        cosT = np.tile(np.ascontiguousarray(cs.T), (2, 1)).astype(BF16)
        sinT = np.tile(np.ascontiguousarray(sn.T) * sgn[:, None],
                       (2, 1)).astype(BF16)
        # bmaskT[kj, q] = 1 where q's tile and kj belong to different segments
        # (both used); the device adds -30000 * bmask to those score columns.
        qt = np.arange(TOK) // 128
        bmask = ((tseg[:, None] != tseg[qt][None, :])
                 & (tseg[:, None] >= 0) & (tseg[qt][None, :] >= 0))
        bmrow = np.zeros((1, bmcols), F32)
        for groups in structure:
            for (_, flat) in groups:
                for (off, kj, qlo, qhi, doff, nq, bm) in flat:
                    if bm is not None:
                        bmrow[0, bm:bm + nq] = bmask[kj, qlo * 128:qlo * 128 + nq]
        per_core.append(dict(tokmap=tokmap,
                             xT=np.ascontiguousarray(x.T).astype(BF16),
                             cosT=cosT, sinT=sinT,
                             bmaskT=bmrow.astype(BF16)))
    return per_core


def _shared_inputs(q_w, q_b, k_w, v_w, v_b, out_w, out_b):
    diagA = np.zeros((128, 256), F32)
    diagA[:, 0:128] = np.triu(np.full((128, 128), NEG, F32), k=1)
    diagA[:, 128:256] = np.eye(128, dtype=F32)
    return {
        "wqT": np.ascontiguousarray(np.asarray(q_w, F32).T).astype(BF16),
        "wkT": np.ascontiguousarray(np.asarray(k_w, F32).T).astype(BF16),
        "wvT": np.ascontiguousarray(np.asarray(v_w, F32).T).astype(BF16),
        "woT": np.ascontiguousarray(np.asarray(out_w, F32).T).astype(BF16),
        "qbT": np.ascontiguousarray(np.asarray(q_b, F32).reshape(8, 128).T),
        "obT": np.ascontiguousarray(np.asarray(out_b, F32).reshape(8, 128).T),
        "vb": np.asarray(v_b, F32).reshape(1, EMBED).astype(BF16),
        "diagA": diagA.astype(BF16),
    }


def kernel(hidden_states, cos, sin, q_w, q_b, k_w, v_w, v_b, out_w, out_b,
           seq_len, max_seqlen):
    global LAST_EXEC_NS
    hidden = np.asarray(hidden_states, F32)
    cos = np.asarray(cos, F32)
    sin = np.asarray(sin, F32)
    seq_lens = [int(v) for v in np.asarray(seq_len)]

    core_chunks, structure = _build_plan(seq_lens)
    if structure not in _CACHE:
        _CACHE[structure] = _emit_program(structure)
    nc = _CACHE[structure]

    per_core = _host_prep(hidden, cos, sin, seq_lens, core_chunks, structure)
    shared = _shared_inputs(q_w, q_b, k_w, v_w, v_b, out_w, out_b)
    in_maps = []
    for c in range(NCORES):
        pc = per_core[c]
        in_maps.append({**shared, "xT": pc["xT"], "cosT": pc["cosT"],
                        "sinT": pc["sinT"], "bmaskT": pc["bmaskT"]})

    trace = os.environ.get("BASS_KERNEL_TRACE", "0") == "1"
    if trace:
        _install_ntff_shim()
    import time as _time
    _t0 = _time.time()
    res = run_bass_kernel_spmd(nc, in_maps, core_ids=list(range(NCORES)),
                               trace=trace)
    LAST_EXEC_NS = res.exec_time_ns
    globals()["LAST_TRACE"] = res.instructions_and_trace
    globals()["LAST_RUN_WALL_S"] = _time.time() - _t0

    T = hidden.shape[0]
    out = np.zeros((T, EMBED), F32)
    for c in range(NCORES):
        tokmap = per_core[c]["tokmap"]
        real = tokmap >= 0
        yT = np.asarray(res.results[c]["yT"], F32)
        out[tokmap[real]] = yT.T[real]
    return out
